# Initial kernel scaffold
#
"""AttnBlock (GroupNorm -> QKV 1x1 -> attention over H -> proj -> residual)
for B=8, C=512, H=2048 on 8 trn2 NeuronCores, data-parallel over batch.

Each core runs the full block for one batch element. All heavy matmuls use
float32r (full PE rate at N=512). No on-chip transposes: v is produced
directly as vT[h, o], and scores are computed transposed (ST[j, i]) so the
probability matrix is already laid out with the contraction dim (j) on
partitions for the second attention matmul.

Walrus limits compute-engine instructions to one sync wait each, while Tile
emits all required waits on the first instruction that needs them. We route
every emitted op through a wrapper that can plant same-engine spare NoOps
directly before it; a post-scheduling pass moves excess waits onto the
spares, and a build fixpoint discovers where spares are needed.
"""

import sys

sys.path.insert(0, "/opt/trn_rl_repo")

import numpy as np

B, C, H = 8, 512, 2048
GROUPS = 32
EPS = 1e-6
P = 128
FD = 512
NCH = C // P  # 4 channel chunks
NJT = H // P  # 16 j-chunks
NIT = H // FD  # 4 i-tiles
GPC = 8  # groups per 128-channel chunk (128/16)
CPG = C // GROUPS  # 16 channels per group
SCL = float(C) ** -0.5

_BUILT = None

# instruction types never subject to the 1-wait redistribution
_EXEMPT = ("InstEventSemaphore", "InstNoOp")

_ENG_ATTR = {
    "EngineType.PE": "tensor",
    "EngineType.DVE": "vector",
    "EngineType.Activation": "scalar",
    "EngineType.Pool": "gpsimd",
    "EngineType.SP": "sync",
}


class _Emitter:
    """Emission wrapper: plants pinned spare NoOps before instructions that
    the previous build iteration found to carry >1 sync wait."""

    def __init__(self, nc, tile_mod, needed):
        self.nc = nc
        self.tile = tile_mod
        self.needed = needed
        self.spare_owner = {}  # spare inst name -> key
        self.inst_key = {}  # real inst name -> key
        self.counters = {}
        self.last = {}

    def __call__(self, engine, method, *args, **kw):
        eng = getattr(self.nc, engine)
        idx = self.counters.get(engine, 0)
        self.counters[engine] = idx + 1
        key = (engine, idx)
        for _ in range(self.needed.get(key, 0) if key[0] != "tail" else 0):
            nop = eng.nop(nofuse=True, hint="sparewait")
            self.spare_owner[nop.ins.name] = key
            if self.last.get(engine) is not None:
                self.tile.add_dep_helper(nop.ins, self.last[engine], False, "pin")
            self.last[engine] = nop.ins
        inst = getattr(eng, method)(*args, **kw)
        self.inst_key[inst.ins.name] = key
        self.last[engine] = inst.ins
        return inst

    def plant_tail(self):
        for (kind, engine), n in self.needed.items():
            if kind != "tail":
                continue
            eng = getattr(self.nc, engine)
            for _ in range(n):
                nop = eng.nop(nofuse=True, hint="tailspare")
                self.spare_owner[nop.ins.name] = ("tail", engine)
                if self.last.get(engine) is not None:
                    self.tile.add_dep_helper(nop.ins, self.last[engine], False, "pin")
                self.last[engine] = nop.ins


def redistribute_waits(nc, em, mybir, max_waits=1):
    """Move excess sync waits onto the spare NoOps planted for each
    instruction (pinned directly before it on the same engine queue).
    Returns dict key -> spare count still needed."""
    by_owner = {}
    deficit = {}
    for blk in nc.m.functions[0].blocks:
        for ins in blk.instructions:
            own = em.spare_owner.get(ins.name)
            if own is not None:
                by_owner.setdefault(own, []).append(ins)
    for blk in nc.m.functions[0].blocks:
        for ins in blk.instructions:
            if ins.name in em.spare_owner:
                continue
            nm = type(ins).__name__
            if nm in _EXEMPT:
                continue
            if nm == "InstDrain":
                key = ("tail", _ENG_ATTR.get(str(ins.engine), "sync"))
            else:
                key = em.inst_key.get(ins.name)
                if key is None:
                    continue
            si = ins.sync_info
            waits = list(si.on_wait) if si is not None else []
            if len(waits) <= max_waits:
                continue
            excess = waits[: len(waits) - max_waits]
            keep = waits[len(waits) - max_waits :]
            mine = [
                s
                for s in by_owner.get(key, [])
                if not (s.sync_info and s.sync_info.on_wait)
            ]
            if len(excess) > len(mine):
                deficit[key] = deficit.get(key, 0) + len(waits) - max_waits
                continue
            for w, sp in zip(excess, mine):
                old = sp.sync_info
                ou = list(old.on_update) if old is not None else []
                sp.sync_info = mybir.SyncInfo(on_wait=[w], on_update=ou)
            ins.sync_info = mybir.SyncInfo(
                on_wait=keep, on_update=list(si.on_update) if si else []
            )
    return deficit


def check_wait_budget(nc, em, mybir, max_waits=1):
    bad = []
    for blk in nc.m.functions[0].blocks:
        for ins in blk.instructions:
            if type(ins).__name__ in _EXEMPT:
                continue
            si = ins.sync_info
            if si is not None and len(si.on_wait) > max_waits:
                bad.append((ins.name, type(ins).__name__, len(si.on_wait)))
    return bad


def _build_nc(needed, has_bq=False):
    import concourse.bass as bass
    import concourse.tile as tile
    from concourse import mybir

    f32 = mybir.dt.float32
    f32r = mybir.dt.float32r

    nc = bass.Bass()

    x_d = nc.dram_tensor("x", [C, H], f32r, kind="ExternalInput")
    # wall[c-chunk] = [MT | wvT | wpT] slabs (MT = (wk.T @ wq).T), packed
    # on the host: scores^T = hn.T @ (Wk.T Wq) @ hn replaces the separate
    # q and k GEMMs.
    wall_d = nc.dram_tensor("wall", [C, 3 * FD], f32r, kind="ExternalInput")
    # consts = [gamma | beta | bp2 | u(=Wk.T bq) | ind_g] columns
    consts_d = nc.dram_tensor("consts", [P, 4 * NCH + GPC], f32, kind="ExternalInput")
    indb_d = nc.dram_tensor("ind_b", [GPC, P], f32, kind="ExternalInput")
    ones_c_d = nc.dram_tensor("ones_col", [P, 1], f32r, kind="ExternalInput")
    ones_r_d = nc.dram_tensor("ones_row", [1, P], f32r, kind="ExternalInput")
    out_d = nc.dram_tensor("out", [C, H], f32, kind="ExternalOutput")

    from contextlib import ExitStack

    with tile.TileContext(nc) as tc, ExitStack() as ctx:
        em = _Emitter(nc, tile, needed)
        res = ctx.enter_context(tc.tile_pool(name="res", bufs=1))
        work = ctx.enter_context(tc.tile_pool(name="work", bufs=2))
        ps_st = ctx.enter_context(tc.tile_pool(name="ps_st", bufs=3, space="PSUM"))
        ps_mm = ctx.enter_context(tc.tile_pool(name="ps_mm", bufs=2, space="PSUM"))
        ps_aux = ctx.enter_context(tc.tile_pool(name="ps_aux", bufs=2, space="PSUM"))
        ps_den = ctx.enter_context(tc.tile_pool(name="ps_den", bufs=1, space="PSUM"))

        # ---- resident SBUF tiles ----
        xh = [res.tile([P, H], f32r, tag=f"xh{t}", name=f"xh{t}") for t in range(NCH)]
        zt = [res.tile([P, H], f32r, tag=f"zt{t}", name=f"zt{t}") for t in range(NCH)]
        ests = [
            res.tile([P, FD], f32r, tag=f"est{j}", name=f"est{j}") for j in range(NJT)
        ]
        vT = [res.tile([P, FD], f32r, tag=f"vT{j}", name=f"vT{j}") for j in range(NJT)]
        wall_s = [
            res.tile([P, 3 * FD], f32r, tag=f"wall{c}", name=f"wall{c}")
            for c in range(NCH)
        ]
        mt_s = [wall_s[c][:, 0 * FD : 1 * FD] for c in range(NCH)]
        wv_s = [wall_s[c][:, 1 * FD : 2 * FD] for c in range(NCH)]
        wp_s = [wall_s[c][:, 2 * FD : 3 * FD] for c in range(NCH)]
        xr = [res.tile([P, FD], f32, tag=f"xr{o}", name=f"xr{o}") for o in range(NCH)]
        h_s = [res.tile([P, FD], f32r, tag=f"hs{c}", name=f"hs{c}") for c in range(NCH)]
        consts_s = res.tile([P, 4 * NCH + GPC], f32, tag="consts")
        gamma_s = consts_s[:, 0 * NCH : 1 * NCH]
        beta_s = consts_s[:, 1 * NCH : 2 * NCH]
        bp2_s = consts_s[:, 2 * NCH : 3 * NCH]
        u_s = consts_s[:, 3 * NCH : 4 * NCH]
        indg_s = consts_s[:, 4 * NCH : 4 * NCH + GPC]
        indb_s = res.tile([GPC, P], f32, tag="indb")
        g_s = res.tile([P, NJT], f32, tag="g_s") if has_bq else None
        ones_c_s = res.tile([P, 1], f32r, tag="ones_c")
        ones_r_s = res.tile([1, P], f32r, tag="ones_r")
        stat = res.tile([P, 2 * NCH], f32, tag="stat")
        stats2 = res.tile([GPC, 2 * NCH], f32, tag="stats2")
        a_s = res.tile([P, NCH], f32, tag="a_s")
        b_s = res.tile([P, NCH], f32, tag="b_s")
        eps_s = res.tile([GPC, 1], f32, tag="eps")

        AF = mybir.ActivationFunctionType
        OP = mybir.AluOpType

        def est(j):
            return ests[j]

        # ---- phase A: loads + groupnorm ----
        # DMA bandwidth is ~420GB/s per sequencer track; split the loads
        # across sync and gpsimd. x (needed first, for groupnorm) leads on
        # both tracks; the wk quarter of the weight wall follows on sync so
        # the k-GEMM can start as soon as groupnorm finishes; the rest of
        # the wall streams afterwards.
        # x0-x2 on the fast HWDGE (sync) track, x3 chunked on the gpsimd
        # track (bn_stats chases it); the weight wall follows on sync with
        # the mt slab (needed first by the z GEMM) leading.
        em("sync", "dma_start", out=xh[0], in_=x_d[0 * P : 1 * P, :])
        em("sync", "dma_start", out=xh[1], in_=x_d[1 * P : 2 * P, :])
        em("sync", "dma_start", out=xh[2], in_=x_d[2 * P : 3 * P, :])
        em("gpsimd", "dma_start", out=consts_s, in_=consts_d[:, :])
        em("gpsimd", "dma_start", out=indb_s, in_=indb_d[:, :])
        em("gpsimd", "dma_start", out=ones_c_s, in_=ones_c_d[:, :])
        em("gpsimd", "dma_start", out=ones_r_s, in_=ones_r_d[:, :])
        for sg in range(4):
            em(
                "gpsimd",
                "dma_start",
                out=xh[3][:, sg * FD : (sg + 1) * FD],
                in_=x_d[3 * P : 4 * P, sg * FD : (sg + 1) * FD],
            )
        for c in range(NCH):
            em(
                "sync",
                "dma_start",
                out=wall_s[c][:, 0:FD],
                in_=wall_d[c * P : (c + 1) * P, 0:FD],
            )
        for c in range(NCH):
            em(
                "sync",
                "dma_start",
                out=wall_s[c][:, FD:],
                in_=wall_d[c * P : (c + 1) * P, FD:],
            )
        em("vector", "memset", eps_s, EPS)

        # per-channel mean / E[x^2] via bn_stats (chasing the x chunks),
        # then cross-partition group reduction via an indicator matmul.
        for t in range(NCH):
            bn6 = work.tile([P, 4, 6], f32, tag="bn6", name="bn6")
            for sg in range(4):
                em(
                    "vector",
                    "bn_stats",
                    out=bn6[:, sg, :],
                    in_=xh[t][:, sg * FD : (sg + 1) * FD].bitcast(f32),
                )
            mv = work.tile([P, 2], f32, tag="mv", name="mv")
            em("vector", "bn_aggr", out=mv, in_=bn6)
            em("vector", "tensor_copy", out=stat[:, t : t + 1], in_=mv[:, 0:1])
            m2 = work.tile([P, 1], f32, tag="m2", name="m2")
            em("vector", "tensor_mul", out=m2, in0=mv[:, 0:1], in1=mv[:, 0:1])
            em(
                "vector",
                "tensor_add",
                out=stat[:, NCH + t : NCH + t + 1],
                in0=mv[:, 1:2],
                in1=m2,
            )
        g_ps = ps_aux.tile([GPC, 2 * NCH], f32, tag="aux", name="gps")
        em("tensor", "matmul", g_ps, lhsT=indg_s, rhs=stat, start=True, stop=True)
        # group mean / E[x^2] (sum over the 16 channels of a group / 16)
        em("vector", "tensor_scalar_mul", stats2[:, 0:NCH], g_ps[:, 0:NCH], 1.0 / CPG)
        et = work.tile([GPC, NCH], f32, tag="et", name="et")
        em("vector", "tensor_scalar_mul", et, g_ps[:, NCH : 2 * NCH], 1.0 / CPG)
        m2g = work.tile([GPC, NCH], f32, tag="m2g", name="m2g")
        em("vector", "tensor_mul", out=m2g, in0=stats2[:, 0:NCH], in1=stats2[:, 0:NCH])
        var_t = work.tile([GPC, NCH], f32, tag="var", name="var")
        em("vector", "tensor_sub", out=var_t, in0=et, in1=m2g)
        srt = work.tile([GPC, NCH], f32, tag="srt", name="srt")
        em("scalar", "activation", out=srt, in_=var_t, func=AF.Sqrt, bias=eps_s)
        em("vector", "reciprocal", out=stats2[:, NCH : 2 * NCH], in_=srt)
        bc_ps = ps_aux.tile([P, 2 * NCH], f32, tag="aux", name="bcps")
        em("tensor", "matmul", bc_ps, lhsT=indb_s, rhs=stats2, start=True, stop=True)
        em("vector", "tensor_mul", out=a_s, in0=bc_ps[:, NCH : 2 * NCH], in1=gamma_s)
        tmp_ma = work.tile([P, NCH], f32, tag="tmp_ma", name="tmp_ma")
        em("vector", "tensor_mul", out=tmp_ma, in0=bc_ps[:, 0:NCH], in1=a_s)
        em("vector", "tensor_sub", out=b_s, in0=beta_s, in1=tmp_ma)
        # hn = a*x + b in place, sg-major so the first k-GEMM groups can
        # start as soon as slice 0 is normalized; alternate DVE/ACT.
        for sg in range(4):
            for t in range(NCH):
                sl = slice(sg * FD, (sg + 1) * FD)
                if t % 2 == 0:
                    em(
                        "vector",
                        "tensor_scalar",
                        out=xh[t][:, sl],
                        in0=xh[t][:, sl].bitcast(f32),
                        scalar1=a_s[:, t : t + 1],
                        scalar2=b_s[:, t : t + 1],
                        op0=OP.mult,
                        op1=OP.add,
                    )
                else:
                    em(
                        "scalar",
                        "activation",
                        out=xh[t][:, sl],
                        in_=xh[t][:, sl].bitcast(f32),
                        func=AF.Identity,
                        scale=a_s[:, t : t + 1],
                        bias=b_s[:, t : t + 1],
                    )

        # ---- phase B: z = M @ hn and vT GEMMs ----
        for a in range(NCH):
            for n in range(NIT):
                ps = ps_mm.tile([P, FD], f32, tag="mm", name="mmps")
                for b in range(NCH):
                    em(
                        "tensor",
                        "matmul",
                        ps,
                        lhsT=mt_s[b][:, a * P : (a + 1) * P],
                        rhs=xh[b][:, n * FD : (n + 1) * FD],
                        start=(b == 0),
                        stop=(b == NCH - 1),
                    )
                em(
                    "scalar",
                    "activation",
                    out=zt[a][:, n * FD : (n + 1) * FD],
                    in_=ps,
                    func=AF.Copy,
                )
        for j in range(NJT):
            ps = ps_mm.tile([P, FD], f32, tag="mm", name="mmps")
            for c in range(NCH):
                em(
                    "tensor",
                    "matmul",
                    ps,
                    lhsT=xh[c][:, j * P : (j + 1) * P],
                    rhs=wv_s[c],
                    start=(c == 0),
                    stop=(c == NCH - 1),
                )
            em("vector", "tensor_copy", out=vT[j], in_=ps)
        if has_bq:
            # g[j] = (Wk.T bq) . hn[:, j], added (scaled) to scores before
            # exp; needed only when bq != 0.
            for j in range(NJT):
                gp = ps_aux.tile([P, 1], f32, tag="aux", name="gps1")
                for c in range(NCH):
                    em(
                        "tensor",
                        "matmul",
                        gp,
                        lhsT=xh[c][:, j * P : (j + 1) * P],
                        rhs=u_s[:, 0:1].bitcast(f32r)
                        if c == 0
                        else u_s[:, c : c + 1].bitcast(f32r),
                        start=(c == 0),
                        stop=(c == NCH - 1),
                    )
                em("vector", "tensor_scalar_mul", g_s[:, j : j + 1], gp, SCL)

        # ---- phase C: attention + proj per i-tile ----
        def xr_fetch(i0):
            for o in range(NCH):
                em(
                    "sync",
                    "dma_start",
                    out=xr[o],
                    in_=x_d[o * P : (o + 1) * P, i0 : i0 + FD].bitcast(f32),
                )

        for it in range(NIT):
            i0 = it * FD
            xr_fetch(i0)
            # scores^T[j, i] -> exp( * C^-0.5), evicted into est(j).
            # The denominator partial sums run elementwise on the DVE
            # (each est tile holds a distinct j-chunk on partitions, so an
            # elementwise tile sum is a valid partial reduction); only one
            # ones-matmul per i-tile remains on the PE.
            den_ps = ps_den.tile([1, FD], f32, tag="den", name="denps")
            dacc = work.tile([P, FD], f32r, tag="dacc", name="dacc")
            for j in range(NJT):
                ps = ps_st.tile([P, FD], f32, tag="st", name="stps")
                for c in range(NCH):
                    em(
                        "tensor",
                        "matmul",
                        ps,
                        lhsT=xh[c][:, j * P : (j + 1) * P],
                        rhs=zt[c][:, i0 : i0 + FD],
                        start=(c == 0),
                        stop=(c == NCH - 1),
                    )
                em(
                    "scalar",
                    "activation",
                    out=est(j),
                    in_=ps,
                    func=AF.Exp,
                    scale=SCL,
                    bias=g_s[:, j : j + 1] if has_bq else 0.0,
                )
                if j == 1:
                    em(
                        "vector",
                        "tensor_add",
                        out=dacc,
                        in0=est(0).bitcast(f32),
                        in1=est(1).bitcast(f32),
                    )
                elif j > 1:
                    em(
                        "vector",
                        "tensor_add",
                        out=dacc,
                        in0=dacc.bitcast(f32),
                        in1=est(j).bitcast(f32),
                    )
            em(
                "tensor",
                "matmul",
                den_ps,
                lhsT=ones_c_s,
                rhs=dacc,
                start=True,
                stop=True,
            )
            r_s = work.tile([1, FD], f32r, tag="r_s", name="r_s")
            with nc.allow_low_precision(reason="f32r rounding of 1/den is fine"):
                em("vector", "reciprocal", out=r_s, in_=den_ps)
            # h_[c, i] = sum_j vT[j, c] * expST[j, i], then * 1/den.
            # The 1/den broadcast matmul (rb) depends on the slow single-lane
            # reciprocal; emit it after the c=0 h_ group so the PE queue is
            # never head-of-line blocked on the reciprocal chain.
            rb_s = work.tile([P, FD], f32, tag="rb_s", name="rb_s")
            for c in range(NCH):
                ps = ps_mm.tile([P, FD], f32, tag="mm", name="mmps")
                for j in range(NJT):
                    em(
                        "tensor",
                        "matmul",
                        ps,
                        lhsT=vT[j][:, c * P : (c + 1) * P],
                        rhs=est(j),
                        start=(j == 0),
                        stop=(j == NJT - 1),
                    )
                if c == 0:
                    rb_ps = ps_aux.tile([P, FD], f32, tag="aux", name="rbps")
                    em(
                        "tensor",
                        "matmul",
                        rb_ps,
                        lhsT=ones_r_s,
                        rhs=r_s,
                        start=True,
                        stop=True,
                    )
                    em("vector", "tensor_copy", out=rb_s, in_=rb_ps)
                em("vector", "tensor_mul", out=h_s[c], in0=ps, in1=rb_s)
            # proj + bias' + residual
            for o in range(NCH):
                ps = ps_mm.tile([P, FD], f32, tag="mm", name="mmps")
                for c in range(NCH):
                    em(
                        "tensor",
                        "matmul",
                        ps,
                        lhsT=wp_s[c][:, o * P : (o + 1) * P],
                        rhs=h_s[c],
                        start=(c == 0),
                        stop=(c == NCH - 1),
                    )
                o_s = work.tile([P, FD], f32, tag="o_s", name="o_s")
                em(
                    "vector",
                    "scalar_tensor_tensor",
                    out=o_s,
                    in0=ps,
                    scalar=bp2_s[:, o : o + 1],
                    in1=xr[o],
                    op0=OP.add,
                    op1=OP.add,
                )
                em(
                    "sync",
                    "dma_start",
                    out=out_d[o * P : (o + 1) * P, i0 : i0 + FD],
                    in_=o_s,
                )
        em.plant_tail()

    from concourse import mybir as _mybir

    deficit = redistribute_waits(nc, em, _mybir)
    return nc, em, deficit


_BUILT_MAP = {}


def get_built(has_bq=False):
    if has_bq not in _BUILT_MAP:
        needed = {}
        deficit = None
        for attempt in range(8):
            nc, em, deficit = _build_nc(dict(needed), has_bq=has_bq)
            if not deficit:
                break
            for key, n in deficit.items():
                needed[key] = max(needed.get(key, 0), n)
        else:
            raise RuntimeError(f"spare-wait fixpoint did not converge: {deficit}")
        from concourse import mybir

        bad = check_wait_budget(nc, em, mybir)
        if bad:
            raise RuntimeError(f"instructions over wait budget: {bad[:10]}")
        _BUILT_MAP[has_bq] = nc
    return _BUILT_MAP[has_bq]


def _host_prep(x, gamma, beta, wq, bq, wk, bk, wv, bv, wp, bp):
    f = np.float32

    def t128(v):  # [512] -> [128, 4] with element (p, t) = v[t*128 + p]
        return np.ascontiguousarray(v.reshape(NCH, P).T.astype(f))

    ind_g = np.zeros((P, GPC), f)
    ind_g[np.arange(P), np.arange(P) // CPG] = 1.0
    ind_b = np.ascontiguousarray(ind_g.T)
    bp2 = wp.astype(f) @ bv.astype(f) + bp.astype(f)
    # M = Wk.T @ Wq folds the q and k GEMMs into one (scores are bilinear in
    # hn); lhsT layout needs MT = M.T. bk cancels in the softmax; bq needs
    # the u-correction path (zero in this problem).
    mt = (wq.astype(np.float64).T @ wk.astype(np.float64)).astype(f)
    u = (wk.astype(np.float64).T @ bq.astype(np.float64)).astype(f)
    wvT = wv.T.astype(f)
    wpT = wp.T.astype(f)
    wall = np.concatenate([mt, wvT, wpT], axis=1)  # [C, 3C]
    consts = np.concatenate(
        [t128(gamma), t128(beta), t128(bp2), t128(u), ind_g], axis=1
    )
    shared = {
        "wall": np.ascontiguousarray(wall),
        "consts": np.ascontiguousarray(consts),
        "ind_b": ind_b,
        "ones_col": np.ones((P, 1), f),
        "ones_row": np.ones((1, P), f),
    }
    return [{"x": np.ascontiguousarray(x[b].astype(f)), **shared} for b in range(B)], bool(
        np.any(bq != 0)
    )


def run(inputs, trace=False, **kw):
    from concourse.bass_utils import run_bass_kernel_spmd

    in_maps, has_bq = _host_prep(**{k: np.asarray(v) for k, v in inputs.items()})
    nc = get_built(has_bq=has_bq)
    res = run_bass_kernel_spmd(nc, in_maps, list(range(B)), trace=trace, **kw)
    out = np.stack([res.results[b]["out"] for b in range(B)]).astype(np.float32)
    return out, res


def kernel(**inputs):
    out, _ = run(inputs, trace=False)
    return out



# revision 7
# speedup vs baseline: 1.4421x; 1.4421x over previous
"""AttnBlock (GroupNorm -> QKV 1x1 -> attention over H -> proj -> residual)
for B=8, C=512, H=2048 on 8 trn2 NeuronCores, data-parallel over batch.

Each core runs the full block for one batch element. All heavy matmuls run
in bf16 (inputs rounded on-chip; PSUM accumulation stays fp32), which keeps
the 1-cycle/row PE rate while enabling fast-weight-load so the per-matmul
LDWEIGHTS cost hides inside the PE reorder window. Raw x stays resident in
fp32 for the group-norm stats and the residual add (no per-tile refetch);
the normalized copy, z (= (Wk^T Wq) @ hn), vT, exp-scores and h_ tiles are
bf16. exp-score tiles are double-buffered across i-tiles so consecutive
tiles pipeline on the PE with no drain between them. Group-norm stats are
split DVE (bn_stats, chunks 0-2) / ACT (Copy+Square accum_out, chunk 3) so
they chase the x DMA instead of serializing after it.

Walrus limits compute-engine instructions to one sync wait each, while Tile
emits all required waits on the first instruction that needs them. We route
every emitted op through a wrapper that can plant same-engine spare NoOps
directly before it; a post-scheduling pass moves excess waits onto the
spares, and a build fixpoint discovers where spares are needed.
"""

import sys

sys.path.insert(0, "/opt/trn_rl_repo")

import numpy as np

B, C, H = 8, 512, 2048
GROUPS = 32
EPS = 1e-6
P = 128
FD = 512
NCH = C // P  # 4 channel chunks
NJT = H // P  # 16 j-chunks
NIT = H // FD  # 4 i-tiles
GPC = 8  # groups per 128-channel chunk (128/16)
CPG = C // GROUPS  # 16 channels per group
SCL = float(C) ** -0.5
ESHIFT = -3.0  # softmax logit shift: keeps exp() in fp8e4 range (cancels in 1/den)

# instruction types never subject to the 1-wait redistribution
_EXEMPT = ("InstEventSemaphore", "InstNoOp")

_ENG_ATTR = {
    "EngineType.PE": "tensor",
    "EngineType.DVE": "vector",
    "EngineType.Activation": "scalar",
    "EngineType.Pool": "gpsimd",
    "EngineType.SP": "sync",
}


class _Emitter:
    """Emission wrapper: plants pinned spare NoOps before instructions that
    the previous build iteration found to carry >1 sync wait."""

    def __init__(self, nc, tile_mod, needed):
        self.nc = nc
        self.tile = tile_mod
        self.needed = needed
        self.spare_owner = {}  # spare inst name -> key
        self.inst_key = {}  # real inst name -> key
        self.counters = {}
        self.last = {}

    def __call__(self, engine, method, *args, **kw):
        eng = getattr(self.nc, engine)
        idx = self.counters.get(engine, 0)
        self.counters[engine] = idx + 1
        key = (engine, idx)
        for _ in range(self.needed.get(key, 0) if key[0] != "tail" else 0):
            nop = eng.nop(nofuse=True, hint="sparewait")
            self.spare_owner[nop.ins.name] = key
            if self.last.get(engine) is not None:
                self.tile.add_dep_helper(nop.ins, self.last[engine], False, "pin")
            self.last[engine] = nop.ins
        inst = getattr(eng, method)(*args, **kw)
        self.inst_key[inst.ins.name] = key
        self.last[engine] = inst.ins
        return inst

    def plant_tail(self):
        for (kind, engine), n in self.needed.items():
            if kind != "tail":
                continue
            eng = getattr(self.nc, engine)
            for _ in range(n):
                nop = eng.nop(nofuse=True, hint="tailspare")
                self.spare_owner[nop.ins.name] = ("tail", engine)
                if self.last.get(engine) is not None:
                    self.tile.add_dep_helper(nop.ins, self.last[engine], False, "pin")
                self.last[engine] = nop.ins


def redistribute_waits(nc, em, mybir, max_waits=1):
    """Move excess sync waits onto the spare NoOps planted for each
    instruction (pinned directly before it on the same engine queue).
    Returns dict key -> spare count still needed."""
    by_owner = {}
    deficit = {}
    for blk in nc.m.functions[0].blocks:
        for ins in blk.instructions:
            own = em.spare_owner.get(ins.name)
            if own is not None:
                by_owner.setdefault(own, []).append(ins)
    for blk in nc.m.functions[0].blocks:
        for ins in blk.instructions:
            if ins.name in em.spare_owner:
                continue
            nm = type(ins).__name__
            if nm in _EXEMPT:
                continue
            if nm == "InstDrain":
                key = ("tail", _ENG_ATTR.get(str(ins.engine), "sync"))
            else:
                key = em.inst_key.get(ins.name)
                if key is None:
                    continue
            si = ins.sync_info
            waits = list(si.on_wait) if si is not None else []
            if len(waits) <= max_waits:
                continue
            excess = waits[: len(waits) - max_waits]
            keep = waits[len(waits) - max_waits :]
            mine = [
                s
                for s in by_owner.get(key, [])
                if not (s.sync_info and s.sync_info.on_wait)
            ]
            if len(excess) > len(mine):
                deficit[key] = deficit.get(key, 0) + len(waits) - max_waits
                continue
            for w, sp in zip(excess, mine):
                old = sp.sync_info
                ou = list(old.on_update) if old is not None else []
                sp.sync_info = mybir.SyncInfo(on_wait=[w], on_update=ou)
            ins.sync_info = mybir.SyncInfo(
                on_wait=keep, on_update=list(si.on_update) if si else []
            )
    return deficit


def check_wait_budget(nc, em, mybir, max_waits=1):
    bad = []
    for blk in nc.m.functions[0].blocks:
        for ins in blk.instructions:
            if type(ins).__name__ in _EXEMPT:
                continue
            si = ins.sync_info
            if si is not None and len(si.on_wait) > max_waits:
                bad.append((ins.name, type(ins).__name__, len(si.on_wait)))
    return bad


def _build_nc(needed, has_bq=False):
    import concourse.bass as bass
    import concourse.tile as tile
    from concourse import mybir

    f32 = mybir.dt.float32
    f32r = mybir.dt.float32r
    bf16 = mybir.dt.bfloat16
    fp8 = mybir.dt.float8e4

    nc = bass.Bass()

    x_d = nc.dram_tensor("x", [C, H], f32, kind="ExternalInput")
    # wall[c-chunk] = [MT | wvT | wpT] slabs (MT = (wk.T @ wq).T), packed
    # on the host in bf16: scores^T = hn.T @ (Wk.T Wq) @ hn replaces the
    # separate q and k GEMMs.
    wall_d = nc.dram_tensor("wall", [C, 3 * FD], bf16, kind="ExternalInput")
    # consts = [gamma | beta | bp2 | u(=Wk.T bq) | ind_g] columns
    consts_d = nc.dram_tensor("consts", [P, 4 * NCH + GPC], f32, kind="ExternalInput")
    indb_d = nc.dram_tensor("ind_b", [GPC, P], f32, kind="ExternalInput")
    ones_c_d = nc.dram_tensor("ones_col", [P, 1], bf16, kind="ExternalInput")
    ones_r_d = nc.dram_tensor("ones_row", [1, P], f32r, kind="ExternalInput")
    out_d = nc.dram_tensor("out", [C, H], f32, kind="ExternalOutput")

    from contextlib import ExitStack

    with tile.TileContext(nc) as tc, ExitStack() as ctx:
        em = _Emitter(nc, tile, needed)
        res = ctx.enter_context(tc.tile_pool(name="res", bufs=1))
        work = ctx.enter_context(tc.tile_pool(name="work", bufs=2))
        ps_st = ctx.enter_context(tc.tile_pool(name="ps_st", bufs=3, space="PSUM"))
        ps_mm = ctx.enter_context(tc.tile_pool(name="ps_mm", bufs=3, space="PSUM"))
        ps_aux = ctx.enter_context(tc.tile_pool(name="ps_aux", bufs=1, space="PSUM"))
        ps_den = ctx.enter_context(tc.tile_pool(name="ps_den", bufs=1, space="PSUM"))
        ost = ctx.enter_context(tc.tile_pool(name="ost", bufs=4))

        # ---- resident SBUF tiles ----
        xh = [res.tile([P, H], f32, tag=f"xh{t}", name=f"xh{t}") for t in range(NCH)]
        hn = [res.tile([P, H], bf16, tag=f"hn{t}", name=f"hn{t}") for t in range(NCH)]
        zt = [res.tile([P, H], bf16, tag=f"zt{t}", name=f"zt{t}") for t in range(NCH)]
        ests = [
            [
                res.tile([P, 2, FD], fp8, tag=f"est{p}_{k}", name=f"est{p}_{k}")
                for k in range(NJT // 2)
            ]
            for p in range(2)
        ]
        vp = [
            res.tile([P, 2, FD], fp8, tag=f"vp{k}", name=f"vp{k}")
            for k in range(NJT // 2)
        ]
        wall_s = [
            res.tile([P, 3 * FD], bf16, tag=f"wall{c}", name=f"wall{c}")
            for c in range(NCH)
        ]
        mt_s = [wall_s[c][:, 0 * FD : 1 * FD] for c in range(NCH)]
        wv_s = [wall_s[c][:, 1 * FD : 2 * FD] for c in range(NCH)]
        wp_s = [wall_s[c][:, 2 * FD : 3 * FD] for c in range(NCH)]
        h_s = [res.tile([P, FD], bf16, tag=f"hs{c}", name=f"hs{c}") for c in range(NCH)]
        consts_s = res.tile([P, 4 * NCH + GPC], f32, tag="consts")
        gamma_s = consts_s[:, 0 * NCH : 1 * NCH]
        beta_s = consts_s[:, 1 * NCH : 2 * NCH]
        bp2_s = consts_s[:, 2 * NCH : 3 * NCH]
        u_s = consts_s[:, 3 * NCH : 4 * NCH]
        indg_s = consts_s[:, 4 * NCH : 4 * NCH + GPC]
        indb_s = res.tile([GPC, P], f32, tag="indb")
        g_s = res.tile([P, NJT], f32, tag="g_s") if has_bq else None
        u_b = res.tile([P, NCH], bf16, tag="u_b") if has_bq else None
        ones_c_s = res.tile([P, 1], bf16, tag="ones_c")
        ones_r_s = res.tile([1, P], f32r, tag="ones_r")
        stat = res.tile([P, 2 * NCH], f32, tag="stat")
        stats2 = res.tile([GPC, 2 * NCH], f32, tag="stats2")
        a_s = res.tile([P, NCH], f32, tag="a_s")
        b_s = res.tile([P, NCH], f32, tag="b_s")
        eps_s = res.tile([GPC, 1], f32, tag="eps")
        esh_s = res.tile([P, 1], f32, tag="esh")
        s3acc = res.tile([P, 2], f32, tag="s3acc")

        AF = mybir.ActivationFunctionType
        OP = mybir.AluOpType

        # ---- phase A: loads + groupnorm stats ----
        # x streams in [P, FD] slabs split across the two fast DMA trigger
        # tracks (sync / gpsimd) so per-chunk stats can chase the transfer:
        # chunks 0-2 feed DVE bn_stats, chunk 3 (issued first on its track)
        # feeds the ACT accum path.
        em("sync", "dma_start", out=xh[0], in_=x_d[0 * P : 1 * P, :])
        em("sync", "dma_start", out=xh[1], in_=x_d[1 * P : 2 * P, :])
        em("gpsimd", "dma_start", out=xh[3], in_=x_d[3 * P : 4 * P, :])
        em("gpsimd", "dma_start", out=xh[2], in_=x_d[2 * P : 3 * P, :])
        # weight wall: mt slab first (z GEMM needs it right after stats)
        for c in range(NCH):
            em("sync", "dma_start", out=wall_s[c][:, 0:FD],
               in_=wall_d[c * P : (c + 1) * P, 0:FD])
        em("gpsimd", "dma_start", out=consts_s, in_=consts_d[:, :])
        em("gpsimd", "dma_start", out=indb_s, in_=indb_d[:, :])
        em("gpsimd", "dma_start", out=ones_c_s, in_=ones_c_d[:, :])
        em("gpsimd", "dma_start", out=ones_r_s, in_=ones_r_d[:, :])
        for c in range(NCH):
            em("sync" if c % 2 == 0 else "gpsimd", "dma_start",
               out=wall_s[c][:, FD:], in_=wall_d[c * P : (c + 1) * P, FD:])
        em("vector", "memset", eps_s, EPS)
        em("vector", "memset", esh_s, ESHIFT)

        # chunks 0-2: per-channel mean / E[x^2] via bn_stats on DVE
        for t in range(3):
            bn6 = work.tile([P, 4, 6], f32, tag="bn6", name="bn6")
            for sg in range(4):
                em("vector", "bn_stats", out=bn6[:, sg, :],
                   in_=xh[t][:, sg * FD : (sg + 1) * FD])
            mv = work.tile([P, 2], f32, tag="mv", name="mv")
            em("vector", "bn_aggr", out=mv, in_=bn6)
            em("vector", "tensor_copy", out=stat[:, t : t + 1], in_=mv[:, 0:1])
            m2 = work.tile([P, 1], f32, tag="m2", name="m2")
            em("vector", "tensor_mul", out=m2, in0=mv[:, 0:1], in1=mv[:, 0:1])
            em("vector", "tensor_add", out=stat[:, NCH + t : NCH + t + 1],
               in0=mv[:, 1:2], in1=m2)
        # chunk 3: sum / sum-of-squares via ACT accum_out (hn[3] is scratch
        # for the full-rate outputs; it gets overwritten by normalize later)
        em("scalar", "activation", out=hn[3], in_=xh[3], func=AF.Copy,
           accum_out=s3acc[:, 0:1])
        em("scalar", "activation", out=hn[3], in_=xh[3], func=AF.Square,
           accum_out=s3acc[:, 1:2])
        em("vector", "tensor_scalar_mul", stat[:, 3:4], s3acc[:, 0:1], 1.0 / H)
        em("vector", "tensor_scalar_mul", stat[:, NCH + 3 : NCH + 4],
           s3acc[:, 1:2], 1.0 / H)

        # cross-partition group reduction via an indicator matmul
        g_ps = ps_aux.tile([GPC, 2 * NCH], f32, tag="aux", name="gps")
        em("tensor", "matmul", g_ps, lhsT=indg_s, rhs=stat, start=True, stop=True)
        # group mean / E[x^2] (sum over the 16 channels of a group / 16)
        em("vector", "tensor_scalar_mul", stats2[:, 0:NCH], g_ps[:, 0:NCH], 1.0 / CPG)
        et = work.tile([GPC, NCH], f32, tag="et", name="et")
        em("vector", "tensor_scalar_mul", et, g_ps[:, NCH : 2 * NCH], 1.0 / CPG)
        m2g = work.tile([GPC, NCH], f32, tag="m2g", name="m2g")
        em("vector", "tensor_mul", out=m2g, in0=stats2[:, 0:NCH], in1=stats2[:, 0:NCH])
        var_t = work.tile([GPC, NCH], f32, tag="var", name="var")
        em("vector", "tensor_sub", out=var_t, in0=et, in1=m2g)
        srt = work.tile([GPC, NCH], f32, tag="srt", name="srt")
        em("scalar", "activation", out=srt, in_=var_t, func=AF.Sqrt, bias=eps_s)
        em("vector", "reciprocal", out=stats2[:, NCH : 2 * NCH], in_=srt)
        bc_ps = ps_aux.tile([P, 2 * NCH], f32, tag="aux", name="bcps")
        em("tensor", "matmul", bc_ps, lhsT=indb_s, rhs=stats2, start=True, stop=True)
        em("vector", "tensor_mul", out=a_s, in0=bc_ps[:, NCH : 2 * NCH], in1=gamma_s)
        tmp_ma = work.tile([P, NCH], f32, tag="tmp_ma", name="tmp_ma")
        em("vector", "tensor_mul", out=tmp_ma, in0=bc_ps[:, 0:NCH], in1=a_s)
        em("vector", "tensor_sub", out=b_s, in0=beta_s, in1=tmp_ma)
        # hn = a*x + b into the bf16 copies, sg-major so the first z-GEMM
        # groups can start as soon as slice 0 is normalized; alternate
        # DVE/ACT. Raw x stays untouched for the residual add.
        for sg in range(4):
            for t in range(NCH):
                sl = slice(sg * FD, (sg + 1) * FD)
                if t % 2 == 0:
                    em(
                        "vector",
                        "tensor_scalar",
                        out=hn[t][:, sl],
                        in0=xh[t][:, sl],
                        scalar1=a_s[:, t : t + 1],
                        scalar2=b_s[:, t : t + 1],
                        op0=OP.mult,
                        op1=OP.add,
                    )
                else:
                    em(
                        "scalar",
                        "activation",
                        out=hn[t][:, sl],
                        in_=xh[t][:, sl],
                        func=AF.Identity,
                        scale=a_s[:, t : t + 1],
                        bias=b_s[:, t : t + 1],
                    )

        # ---- phase B: z = M @ hn and vT GEMMs ----
        for a in range(NCH):
            for n in range(NIT):
                ps = ps_mm.tile([P, FD], f32, tag="mm", name="mmps")
                for b in range(NCH):
                    em(
                        "tensor",
                        "matmul",
                        ps,
                        lhsT=mt_s[b][:, a * P : (a + 1) * P],
                        rhs=hn[b][:, n * FD : (n + 1) * FD],
                        start=(b == 0),
                        stop=(b == NCH - 1),
                    )
                em(
                    "scalar",
                    "activation",
                    out=zt[a][:, n * FD : (n + 1) * FD],
                    in_=ps,
                    func=AF.Copy,
                )
        for j in range(NJT):
            ps = ps_mm.tile([P, FD], f32, tag="mm", name="mmps")
            for c in range(NCH):
                em(
                    "tensor",
                    "matmul",
                    ps,
                    lhsT=hn[c][:, j * P : (j + 1) * P],
                    rhs=wv_s[c],
                    start=(c == 0),
                    stop=(c == NCH - 1),
                )
            if j % 2 == 0:
                em("vector", "tensor_copy", out=vp[j // 2][:, 0, :], in_=ps)
            else:
                em("scalar", "activation", out=vp[j // 2][:, 1, :], in_=ps,
                   func=AF.Copy)
        if has_bq:
            # g[j] = (Wk.T bq) . hn[:, j], added (scaled) to scores before
            # exp; needed only when bq != 0.
            em("vector", "tensor_copy", out=u_b, in_=u_s)
            for j in range(NJT):
                gp = ps_aux.tile([P, 1], f32, tag="aux", name="gps1")
                for c in range(NCH):
                    em(
                        "tensor",
                        "matmul",
                        gp,
                        lhsT=hn[c][:, j * P : (j + 1) * P],
                        rhs=u_b[:, c : c + 1],
                        start=(c == 0),
                        stop=(c == NCH - 1),
                    )
                em("vector", "tensor_scalar", out=g_s[:, j : j + 1], in0=gp,
                   scalar1=SCL, scalar2=ESHIFT, op0=OP.mult, op1=OP.add)

        # ---- phase C: attention + proj per i-tile ----
        for it in range(NIT):
            i0 = it * FD
            est = ests[it % 2]
            # scores^T[j, i] -> exp( * C^-0.5), evicted into est[j] (bf16).
            # Denominator partial sums run elementwise on the DVE in bf16
            # (each est tile holds a distinct j-chunk on partitions, so an
            # elementwise tile sum is a valid partial reduction); the bf16
            # rounding noise averages out across the fp32 ones-matmul
            # reduction over partitions.
            dacc = work.tile([P, FD], bf16, tag="dacc", name="dacc")
            for j in range(NJT):
                ps = ps_st.tile([P, FD], f32, tag="st", name="stps")
                for c in range(NCH):
                    em(
                        "tensor",
                        "matmul",
                        ps,
                        lhsT=hn[c][:, j * P : (j + 1) * P],
                        rhs=zt[c][:, i0 : i0 + FD],
                        start=(c == 0),
                        stop=(c == NCH - 1),
                    )
                em(
                    "scalar",
                    "activation",
                    out=est[j // 2][:, j % 2, :],
                    in_=ps,
                    func=AF.Exp,
                    scale=SCL,
                    bias=g_s[:, j : j + 1] if has_bq else esh_s,
                )
                if j == 1:
                    em("vector", "tensor_add", out=dacc,
                       in0=est[0][:, 0, :], in1=est[0][:, 1, :])
                elif j > 1:
                    em("vector", "tensor_add", out=dacc,
                       in0=dacc, in1=est[j // 2][:, j % 2, :])
            # h_[c, i] = sum_j vT[j, c] * expST[j, i], then * 1/den.
            # The den and rb matmuls are interleaved after the c=0 / c=1
            # attnV groups so the PE queue never head-of-line blocks on the
            # DVE dacc chain or the reciprocal.
            den_ps = ps_den.tile([1, FD], f32, tag="den", name="denps")
            r_s = work.tile([1, FD], f32r, tag="r_s", name="r_s")
            rb_s = work.tile([P, FD], f32, tag="rb_s", name="rb_s")
            ps0 = None
            for c in range(NCH):
                ps = ps_mm.tile([P, FD], f32, tag="mm", name="mmps")
                for k in range(NJT // 2):
                    em(
                        "tensor",
                        "matmul",
                        ps,
                        lhsT=vp[k][:, :, c * P : (c + 1) * P],
                        rhs=est[k],
                        start=(k == 0),
                        stop=(k == NJT // 2 - 1),
                        perf_mode=mybir.MatmulPerfMode.DoubleRow,
                    )
                if c == 0:
                    em(
                        "tensor",
                        "matmul",
                        den_ps,
                        lhsT=ones_c_s,
                        rhs=dacc,
                        start=True,
                        stop=True,
                    )
                    with nc.allow_low_precision(reason="f32r rounding of 1/den"):
                        em("vector", "reciprocal", out=r_s, in_=den_ps)
                    ps0 = ps  # h_mul for c=0 is deferred until rb_s exists
                    continue
                if c == 1:
                    rb_ps = ps_aux.tile([P, FD], f32, tag="aux", name="rbps")
                    em(
                        "tensor",
                        "matmul",
                        rb_ps,
                        lhsT=ones_r_s,
                        rhs=r_s,
                        start=True,
                        stop=True,
                    )
                    em("vector", "tensor_copy", out=rb_s, in_=rb_ps)
                    em("vector", "tensor_mul", out=h_s[0], in0=ps0, in1=rb_s)
                em("vector", "tensor_mul", out=h_s[c], in0=ps, in1=rb_s)
            # proj + bias' + residual (straight from the resident raw x)
            for o in range(NCH):
                ps = ps_mm.tile([P, FD], f32, tag="mm", name="mmps")
                for c in range(NCH):
                    em(
                        "tensor",
                        "matmul",
                        ps,
                        lhsT=wp_s[c][:, o * P : (o + 1) * P],
                        rhs=h_s[c],
                        start=(c == 0),
                        stop=(c == NCH - 1),
                    )
                o_s = ost.tile([P, FD], f32, tag="o_s", name="o_s")
                em(
                    "vector",
                    "scalar_tensor_tensor",
                    out=o_s,
                    in0=ps,
                    scalar=bp2_s[:, o : o + 1],
                    in1=xh[o][:, i0 : i0 + FD],
                    op0=OP.add,
                    op1=OP.add,
                )
                em(
                    "sync",
                    "dma_start",
                    out=out_d[o * P : (o + 1) * P, i0 : i0 + FD],
                    in_=o_s,
                )
        em.plant_tail()

    from concourse import mybir as _mybir

    deficit = redistribute_waits(nc, em, _mybir)
    return nc, em, deficit


_BUILT_MAP = {}


def get_built(has_bq=False):
    if has_bq not in _BUILT_MAP:
        needed = {}
        deficit = None
        for attempt in range(8):
            nc, em, deficit = _build_nc(dict(needed), has_bq=has_bq)
            if not deficit:
                break
            for key, n in deficit.items():
                needed[key] = max(needed.get(key, 0), n)
        else:
            raise RuntimeError(f"spare-wait fixpoint did not converge: {deficit}")
        from concourse import mybir

        bad = check_wait_budget(nc, em, mybir)
        if bad:
            raise RuntimeError(f"instructions over wait budget: {bad[:10]}")
        _BUILT_MAP[has_bq] = nc
    return _BUILT_MAP[has_bq]


def _host_prep(x, gamma, beta, wq, bq, wk, bk, wv, bv, wp, bp):
    import ml_dtypes

    f = np.float32
    bf = ml_dtypes.bfloat16

    def t128(v):  # [512] -> [128, 4] with element (p, t) = v[t*128 + p]
        return np.ascontiguousarray(v.reshape(NCH, P).T.astype(f))

    ind_g = np.zeros((P, GPC), f)
    ind_g[np.arange(P), np.arange(P) // CPG] = 1.0
    ind_b = np.ascontiguousarray(ind_g.T)
    bp2 = wp.astype(f) @ bv.astype(f) + bp.astype(f)
    # M = Wk.T @ Wq folds the q and k GEMMs into one (scores are bilinear in
    # hn); lhsT layout needs MT = M.T. bk cancels in the softmax; bq needs
    # the u-correction path (zero in this problem).
    mt = (wq.astype(np.float64).T @ wk.astype(np.float64)).astype(f)
    u = (wk.astype(np.float64).T @ bq.astype(np.float64)).astype(f)
    wvT = wv.T.astype(f)
    wpT = wp.T.astype(f)
    wall = np.concatenate([mt, wvT, wpT], axis=1).astype(bf)  # [C, 3C]
    consts = np.concatenate(
        [t128(gamma), t128(beta), t128(bp2), t128(u), ind_g], axis=1
    )
    shared = {
        "wall": np.ascontiguousarray(wall),
        "consts": np.ascontiguousarray(consts),
        "ind_b": ind_b,
        "ones_col": np.ones((P, 1), bf),
        "ones_row": np.ones((1, P), f),
    }
    return [{"x": np.ascontiguousarray(x[b].astype(f)), **shared} for b in range(B)], bool(
        np.any(bq != 0)
    )


def run(inputs, trace=False, **kw):
    from concourse.bass_utils import run_bass_kernel_spmd

    in_maps, has_bq = _host_prep(**{k: np.asarray(v) for k, v in inputs.items()})
    nc = get_built(has_bq=has_bq)
    res = run_bass_kernel_spmd(nc, in_maps, list(range(B)), trace=trace, **kw)
    out = np.stack([res.results[b]["out"] for b in range(B)]).astype(np.float32)
    return out, res


def kernel(**inputs):
    out, _ = run(inputs, trace=False)
    return out


# revision 9
# speedup vs baseline: 1.5283x; 1.0598x over previous
"""AttnBlock (GroupNorm -> QKV 1x1 -> attention over H -> proj -> residual)
for B=8, C=512, H=2048 on 8 trn2 NeuronCores, data-parallel over batch.

Each core runs the full block for one batch element. All heavy matmuls run
in bf16 (inputs rounded on-chip; PSUM accumulation stays fp32), which keeps
the 1-cycle/row PE rate while enabling fast-weight-load so the per-matmul
LDWEIGHTS cost hides inside the PE reorder window. Raw x stays resident in
fp32 for the group-norm stats and the residual add (no per-tile refetch);
the normalized copy, z (= (Wk^T Wq) @ hn), vT, exp-scores and h_ tiles are
bf16. exp-score tiles are double-buffered across i-tiles so consecutive
tiles pipeline on the PE with no drain between them. Group-norm stats are
split DVE (bn_stats, chunks 0-2) / ACT (Copy+Square accum_out, chunk 3) so
they chase the x DMA instead of serializing after it.

Walrus limits compute-engine instructions to one sync wait each, while Tile
emits all required waits on the first instruction that needs them. We route
every emitted op through a wrapper that can plant same-engine spare NoOps
directly before it; a post-scheduling pass moves excess waits onto the
spares, and a build fixpoint discovers where spares are needed.
"""

import sys

sys.path.insert(0, "/opt/trn_rl_repo")

import numpy as np

B, C, H = 8, 512, 2048
GROUPS = 32
EPS = 1e-6
P = 128
FD = 512
NCH = C // P  # 4 channel chunks
NJT = H // P  # 16 j-chunks
NIT = H // FD  # 4 i-tiles
GPC = 8  # groups per 128-channel chunk (128/16)
CPG = C // GROUPS  # 16 channels per group
SCL = float(C) ** -0.5
ESHIFT = -3.0  # softmax logit shift: keeps exp() in fp8e4 range (cancels in 1/den)

# instruction types never subject to the 1-wait redistribution
_EXEMPT = ("InstEventSemaphore", "InstNoOp")

_ENG_ATTR = {
    "EngineType.PE": "tensor",
    "EngineType.DVE": "vector",
    "EngineType.Activation": "scalar",
    "EngineType.Pool": "gpsimd",
    "EngineType.SP": "sync",
}


class _Emitter:
    """Emission wrapper: plants pinned spare NoOps before instructions that
    the previous build iteration found to carry >1 sync wait."""

    def __init__(self, nc, tile_mod, needed):
        self.nc = nc
        self.tile = tile_mod
        self.needed = needed
        self.spare_owner = {}  # spare inst name -> key
        self.inst_key = {}  # real inst name -> key
        self.counters = {}
        self.last = {}

    def __call__(self, engine, method, *args, **kw):
        eng = getattr(self.nc, engine)
        idx = self.counters.get(engine, 0)
        self.counters[engine] = idx + 1
        key = (engine, idx)
        for _ in range(self.needed.get(key, 0) if key[0] != "tail" else 0):
            nop = eng.nop(nofuse=True, hint="sparewait")
            self.spare_owner[nop.ins.name] = key
            if self.last.get(engine) is not None:
                self.tile.add_dep_helper(nop.ins, self.last[engine], False, "pin")
            self.last[engine] = nop.ins
        inst = getattr(eng, method)(*args, **kw)
        self.inst_key[inst.ins.name] = key
        self.last[engine] = inst.ins
        return inst

    def plant_tail(self):
        for (kind, engine), n in self.needed.items():
            if kind != "tail":
                continue
            eng = getattr(self.nc, engine)
            for _ in range(n):
                nop = eng.nop(nofuse=True, hint="tailspare")
                self.spare_owner[nop.ins.name] = ("tail", engine)
                if self.last.get(engine) is not None:
                    self.tile.add_dep_helper(nop.ins, self.last[engine], False, "pin")
                self.last[engine] = nop.ins


def redistribute_waits(nc, em, mybir, max_waits=1):
    """Move excess sync waits onto the spare NoOps planted for each
    instruction (pinned directly before it on the same engine queue).
    Returns dict key -> spare count still needed."""
    by_owner = {}
    deficit = {}
    for blk in nc.m.functions[0].blocks:
        for ins in blk.instructions:
            own = em.spare_owner.get(ins.name)
            if own is not None:
                by_owner.setdefault(own, []).append(ins)
    for blk in nc.m.functions[0].blocks:
        for ins in blk.instructions:
            if ins.name in em.spare_owner:
                continue
            nm = type(ins).__name__
            if nm in _EXEMPT:
                continue
            if nm == "InstDrain":
                key = ("tail", _ENG_ATTR.get(str(ins.engine), "sync"))
            else:
                key = em.inst_key.get(ins.name)
                if key is None:
                    continue
            si = ins.sync_info
            waits = list(si.on_wait) if si is not None else []
            if len(waits) <= max_waits:
                continue
            excess = waits[: len(waits) - max_waits]
            keep = waits[len(waits) - max_waits :]
            mine = [
                s
                for s in by_owner.get(key, [])
                if not (s.sync_info and s.sync_info.on_wait)
            ]
            if len(excess) > len(mine):
                deficit[key] = deficit.get(key, 0) + len(waits) - max_waits
                continue
            for w, sp in zip(excess, mine):
                old = sp.sync_info
                ou = list(old.on_update) if old is not None else []
                sp.sync_info = mybir.SyncInfo(on_wait=[w], on_update=ou)
            ins.sync_info = mybir.SyncInfo(
                on_wait=keep, on_update=list(si.on_update) if si else []
            )
    return deficit


def check_wait_budget(nc, em, mybir, max_waits=1):
    bad = []
    for blk in nc.m.functions[0].blocks:
        for ins in blk.instructions:
            if type(ins).__name__ in _EXEMPT:
                continue
            si = ins.sync_info
            if si is not None and len(si.on_wait) > max_waits:
                bad.append((ins.name, type(ins).__name__, len(si.on_wait)))
    return bad


def _build_nc(needed, has_bq=False):
    import concourse.bass as bass
    import concourse.tile as tile
    from concourse import mybir

    f32 = mybir.dt.float32
    f32r = mybir.dt.float32r
    bf16 = mybir.dt.bfloat16
    fp8 = mybir.dt.float8e4

    nc = bass.Bass()

    x_d = nc.dram_tensor("x", [C, H], f32, kind="ExternalInput")
    xb_d = nc.dram_tensor("xb", [C, H], bf16, kind="ExternalInput")
    # wall[c-chunk] = [MT | wvT | wpT] slabs (MT = (wk.T @ wq).T), packed
    # on the host in bf16: scores^T = hn.T @ (Wk.T Wq) @ hn replaces the
    # separate q and k GEMMs.
    wall_d = nc.dram_tensor("wall", [C, 3 * FD], bf16, kind="ExternalInput")
    # consts = [gamma | beta | bp2 | u(=Wk.T bq) | ind_g] columns
    consts_d = nc.dram_tensor("consts", [P, 4 * NCH + GPC], f32, kind="ExternalInput")
    indb_d = nc.dram_tensor("ind_b", [GPC, P], f32, kind="ExternalInput")
    ones_c_d = nc.dram_tensor("ones_col", [P, 1], bf16, kind="ExternalInput")
    ones_r_d = nc.dram_tensor("ones_row", [1, P], f32r, kind="ExternalInput")
    out_d = nc.dram_tensor("out", [C, H], f32, kind="ExternalOutput")

    from contextlib import ExitStack

    with tile.TileContext(nc) as tc, ExitStack() as ctx:
        em = _Emitter(nc, tile, needed)
        res = ctx.enter_context(tc.tile_pool(name="res", bufs=1))
        work = ctx.enter_context(tc.tile_pool(name="work", bufs=2))
        ps_main = ctx.enter_context(tc.tile_pool(name="ps_main", bufs=6, space="PSUM"))
        ps_aux = ctx.enter_context(tc.tile_pool(name="ps_aux", bufs=1, space="PSUM"))
        ps_den = ctx.enter_context(tc.tile_pool(name="ps_den", bufs=1, space="PSUM"))
        ost = ctx.enter_context(tc.tile_pool(name="ost", bufs=4))

        # ---- resident SBUF tiles ----
        xh = [res.tile([P, H], f32, tag=f"xh{t}", name=f"xh{t}") for t in range(NCH)]
        xb = [res.tile([P, H], bf16, tag=f"xb{t}", name=f"xb{t}") for t in range(NCH)]
        hn = [res.tile([P, H], bf16, tag=f"hn{t}", name=f"hn{t}") for t in range(NCH)]
        zt = [res.tile([P, H], bf16, tag=f"zt{t}", name=f"zt{t}") for t in range(NCH)]
        ests = [
            [
                res.tile([P, 2, FD], fp8, tag=f"est{p}_{k}", name=f"est{p}_{k}")
                for k in range(NJT // 2)
            ]
            for p in range(2)
        ]
        vp = [
            res.tile([P, 2, FD], fp8, tag=f"vp{k}", name=f"vp{k}")
            for k in range(NJT // 2)
        ]
        wall_s = [
            res.tile([P, 3 * FD], bf16, tag=f"wall{c}", name=f"wall{c}")
            for c in range(NCH)
        ]
        mt_s = [wall_s[c][:, 0 * FD : 1 * FD] for c in range(NCH)]
        wv_s = [wall_s[c][:, 1 * FD : 2 * FD] for c in range(NCH)]
        wp_s = [wall_s[c][:, 2 * FD : 3 * FD] for c in range(NCH)]
        h_s = [res.tile([P, FD], bf16, tag=f"hs{c}", name=f"hs{c}") for c in range(NCH)]
        consts_s = res.tile([P, 4 * NCH + GPC], f32, tag="consts")
        gamma_s = consts_s[:, 0 * NCH : 1 * NCH]
        beta_s = consts_s[:, 1 * NCH : 2 * NCH]
        bp2_s = consts_s[:, 2 * NCH : 3 * NCH]
        u_s = consts_s[:, 3 * NCH : 4 * NCH]
        indg_s = consts_s[:, 4 * NCH : 4 * NCH + GPC]
        indb_s = res.tile([GPC, P], f32, tag="indb")
        g_s = res.tile([P, NJT], f32, tag="g_s") if has_bq else None
        u_b = res.tile([P, NCH], bf16, tag="u_b") if has_bq else None
        ones_c_s = res.tile([P, 1], bf16, tag="ones_c")
        ones_r_s = res.tile([1, P], f32r, tag="ones_r")
        stat = res.tile([P, 2 * NCH], f32, tag="stat")
        stats2 = res.tile([GPC, 2 * NCH], f32, tag="stats2")
        a_s = res.tile([P, NCH], f32, tag="a_s")
        b_s = res.tile([P, NCH], f32, tag="b_s")
        eps_s = res.tile([GPC, 1], f32, tag="eps")
        esh_s = res.tile([P, 1], f32, tag="esh")
        s3acc = res.tile([P, 2], f32, tag="s3acc")

        AF = mybir.ActivationFunctionType
        OP = mybir.AluOpType

        # ---- phase A: loads + groupnorm stats ----
        # x streams in [P, FD] slabs split across the two fast DMA trigger
        # tracks (sync / gpsimd) so per-chunk stats can chase the transfer:
        # chunks 0-2 feed DVE bn_stats, chunk 3 (issued first on its track)
        # feeds the ACT accum path.
        # the bf16 pre-cast copy of x leads on both tracks: it alone gates
        # stats + normalize + the z GEMM. The f32 x (residual only, first
        # needed ~30us later) and the weight wall stream in behind it.
        em("sync", "dma_start", out=xb[0], in_=xb_d[0 * P : 1 * P, :])
        em("sync", "dma_start", out=xb[1], in_=xb_d[1 * P : 2 * P, :])
        em("gpsimd", "dma_start", out=xb[3], in_=xb_d[3 * P : 4 * P, :])
        em("gpsimd", "dma_start", out=xb[2], in_=xb_d[2 * P : 3 * P, :])
        # weight wall: mt slab first (z GEMM needs it right after stats)
        for c in range(NCH):
            em("sync", "dma_start", out=wall_s[c][:, 0:FD],
               in_=wall_d[c * P : (c + 1) * P, 0:FD])
        em("gpsimd", "dma_start", out=consts_s, in_=consts_d[:, :])
        em("gpsimd", "dma_start", out=indb_s, in_=indb_d[:, :])
        em("gpsimd", "dma_start", out=ones_c_s, in_=ones_c_d[:, :])
        em("gpsimd", "dma_start", out=ones_r_s, in_=ones_r_d[:, :])
        em("sync", "dma_start", out=xh[0], in_=x_d[0 * P : 1 * P, :])
        em("sync", "dma_start", out=xh[1], in_=x_d[1 * P : 2 * P, :])
        em("gpsimd", "dma_start", out=xh[3], in_=x_d[3 * P : 4 * P, :])
        em("gpsimd", "dma_start", out=xh[2], in_=x_d[2 * P : 3 * P, :])
        for c in range(NCH):
            em("sync" if c % 2 == 0 else "gpsimd", "dma_start",
               out=wall_s[c][:, FD:], in_=wall_d[c * P : (c + 1) * P, FD:])
        em("vector", "memset", eps_s, EPS)
        em("vector", "memset", esh_s, ESHIFT)

        # chunks 0-2: per-channel mean / E[x^2] via bn_stats on DVE
        for t in range(3):
            bn6 = work.tile([P, 4, 6], f32, tag="bn6", name="bn6")
            for sg in range(4):
                em("vector", "bn_stats", out=bn6[:, sg, :],
                   in_=xb[t][:, sg * FD : (sg + 1) * FD])
            mv = work.tile([P, 2], f32, tag="mv", name="mv")
            em("vector", "bn_aggr", out=mv, in_=bn6)
            em("vector", "tensor_copy", out=stat[:, t : t + 1], in_=mv[:, 0:1])
            m2 = work.tile([P, 1], f32, tag="m2", name="m2")
            em("vector", "tensor_mul", out=m2, in0=mv[:, 0:1], in1=mv[:, 0:1])
            em("vector", "tensor_add", out=stat[:, NCH + t : NCH + t + 1],
               in0=mv[:, 1:2], in1=m2)
        # chunk 3: sum / sum-of-squares via ACT accum_out (hn[3] is scratch
        # for the full-rate outputs; it gets overwritten by normalize later)
        em("scalar", "activation", out=hn[3], in_=xb[3], func=AF.Copy,
           accum_out=s3acc[:, 0:1])
        em("scalar", "activation", out=hn[3], in_=xb[3], func=AF.Square,
           accum_out=s3acc[:, 1:2])
        em("vector", "tensor_scalar_mul", stat[:, 3:4], s3acc[:, 0:1], 1.0 / H)
        em("vector", "tensor_scalar_mul", stat[:, NCH + 3 : NCH + 4],
           s3acc[:, 1:2], 1.0 / H)

        # cross-partition group reduction via an indicator matmul
        g_ps = ps_aux.tile([GPC, 2 * NCH], f32, tag="aux", name="gps")
        em("tensor", "matmul", g_ps, lhsT=indg_s, rhs=stat, start=True, stop=True)
        # group mean / E[x^2] (sum over the 16 channels of a group / 16)
        em("vector", "tensor_scalar_mul", stats2[:, 0:NCH], g_ps[:, 0:NCH], 1.0 / CPG)
        et = work.tile([GPC, NCH], f32, tag="et", name="et")
        em("vector", "tensor_scalar_mul", et, g_ps[:, NCH : 2 * NCH], 1.0 / CPG)
        m2g = work.tile([GPC, NCH], f32, tag="m2g", name="m2g")
        em("vector", "tensor_mul", out=m2g, in0=stats2[:, 0:NCH], in1=stats2[:, 0:NCH])
        var_t = work.tile([GPC, NCH], f32, tag="var", name="var")
        em("vector", "tensor_sub", out=var_t, in0=et, in1=m2g)
        srt = work.tile([GPC, NCH], f32, tag="srt", name="srt")
        em("scalar", "activation", out=srt, in_=var_t, func=AF.Sqrt, bias=eps_s)
        em("vector", "reciprocal", out=stats2[:, NCH : 2 * NCH], in_=srt)
        bc_ps = ps_aux.tile([P, 2 * NCH], f32, tag="aux", name="bcps")
        em("tensor", "matmul", bc_ps, lhsT=indb_s, rhs=stats2, start=True, stop=True)
        em("vector", "tensor_mul", out=a_s, in0=bc_ps[:, NCH : 2 * NCH], in1=gamma_s)
        tmp_ma = work.tile([P, NCH], f32, tag="tmp_ma", name="tmp_ma")
        em("vector", "tensor_mul", out=tmp_ma, in0=bc_ps[:, 0:NCH], in1=a_s)
        em("vector", "tensor_sub", out=b_s, in0=beta_s, in1=tmp_ma)
        # hn = a*x + b into the bf16 copies, sg-major so the first z-GEMM
        # groups can start as soon as slice 0 is normalized; alternate
        # DVE/ACT. Raw x stays untouched for the residual add.
        for sg in range(4):
            for t in range(NCH):
                sl = slice(sg * FD, (sg + 1) * FD)
                if t % 2 == 0:
                    em(
                        "vector",
                        "tensor_scalar",
                        out=hn[t][:, sl],
                        in0=xb[t][:, sl],
                        scalar1=a_s[:, t : t + 1],
                        scalar2=b_s[:, t : t + 1],
                        op0=OP.mult,
                        op1=OP.add,
                    )
                else:
                    em(
                        "scalar",
                        "activation",
                        out=hn[t][:, sl],
                        in_=xb[t][:, sl],
                        func=AF.Identity,
                        scale=a_s[:, t : t + 1],
                        bias=b_s[:, t : t + 1],
                    )

        # ---- phase B: z = M @ hn and vT GEMMs ----
        for a in range(NCH):
            for n in range(NIT):
                ps = ps_main.tile([P, FD], f32, tag="ps", name="mps")
                for b in range(NCH):
                    em(
                        "tensor",
                        "matmul",
                        ps,
                        lhsT=mt_s[b][:, a * P : (a + 1) * P],
                        rhs=hn[b][:, n * FD : (n + 1) * FD],
                        start=(b == 0),
                        stop=(b == NCH - 1),
                    )
                em(
                    "scalar",
                    "activation",
                    out=zt[a][:, n * FD : (n + 1) * FD],
                    in_=ps,
                    func=AF.Copy,
                )
        for j in range(NJT):
            ps = ps_main.tile([P, FD], f32, tag="ps", name="mps")
            for c in range(NCH):
                em(
                    "tensor",
                    "matmul",
                    ps,
                    lhsT=hn[c][:, j * P : (j + 1) * P],
                    rhs=wv_s[c],
                    start=(c == 0),
                    stop=(c == NCH - 1),
                )
            if j % 2 == 0:
                em("vector", "tensor_copy", out=vp[j // 2][:, 0, :], in_=ps)
            else:
                em("scalar", "activation", out=vp[j // 2][:, 1, :], in_=ps,
                   func=AF.Copy)
        if has_bq:
            # g[j] = (Wk.T bq) . hn[:, j], added (scaled) to scores before
            # exp; needed only when bq != 0.
            em("vector", "tensor_copy", out=u_b, in_=u_s)
            for j in range(NJT):
                gp = ps_aux.tile([P, 1], f32, tag="aux", name="gps1")
                for c in range(NCH):
                    em(
                        "tensor",
                        "matmul",
                        gp,
                        lhsT=hn[c][:, j * P : (j + 1) * P],
                        rhs=u_b[:, c : c + 1],
                        start=(c == 0),
                        stop=(c == NCH - 1),
                    )
                em("vector", "tensor_scalar", out=g_s[:, j : j + 1], in0=gp,
                   scalar1=SCL, scalar2=ESHIFT, op0=OP.mult, op1=OP.add)

        # ---- phase C: attention + proj per i-tile ----
        for it in range(NIT):
            i0 = it * FD
            est = ests[it % 2]
            # scores^T[j, i] -> exp( * C^-0.5), evicted into est[j] (bf16).
            # Denominator partial sums run elementwise on the DVE in bf16
            # (each est tile holds a distinct j-chunk on partitions, so an
            # elementwise tile sum is a valid partial reduction); the bf16
            # rounding noise averages out across the fp32 ones-matmul
            # reduction over partitions.
            dacc = work.tile([P, FD], bf16, tag="dacc", name="dacc")
            for j in range(NJT):
                ps = ps_main.tile([P, FD], f32, tag="ps", name="mps")
                for c in range(NCH):
                    em(
                        "tensor",
                        "matmul",
                        ps,
                        lhsT=hn[c][:, j * P : (j + 1) * P],
                        rhs=zt[c][:, i0 : i0 + FD],
                        start=(c == 0),
                        stop=(c == NCH - 1),
                    )
                em(
                    "scalar",
                    "activation",
                    out=est[j // 2][:, j % 2, :],
                    in_=ps,
                    func=AF.Exp,
                    scale=SCL,
                    bias=g_s[:, j : j + 1] if has_bq else esh_s,
                )
                if j == 1:
                    em("vector", "tensor_add", out=dacc,
                       in0=est[0][:, 0, :], in1=est[0][:, 1, :])
                elif j > 1:
                    em("vector", "tensor_add", out=dacc,
                       in0=dacc, in1=est[j // 2][:, j % 2, :])
            # h_[c, i] = sum_j vT[j, c] * expST[j, i], then * 1/den.
            # The den and rb matmuls are interleaved after the c=0 / c=1
            # attnV groups so the PE queue never head-of-line blocks on the
            # DVE dacc chain or the reciprocal.
            # den -> 1/den -> row-broadcast, pipelined in 128-wide chunks
            # between the attnV groups so neither the single-lane reciprocal
            # nor the broadcast matmul ever head-of-line blocks the PE.
            den_ps = ps_den.tile([1, FD], f32, tag="den", name="denps")
            r_s = work.tile([1, FD], f32r, tag="r_s", name="r_s")
            rb_s = work.tile([P, FD], f32, tag="rb_s", name="rb_s")
            rb_ps = ps_aux.tile([P, FD], f32, tag="aux", name="rbps")
            ps_c = []

            def rb_chunk(q):
                sl = slice(q * P, (q + 1) * P)
                em("tensor", "matmul", rb_ps[:, sl], lhsT=ones_r_s,
                   rhs=r_s[:, sl], start=True, stop=True)

            for c in range(NCH):
                ps = ps_main.tile([P, FD], f32, tag="ps", name="mps")
                ps_c.append(ps)
                for k in range(NJT // 2):
                    em(
                        "tensor",
                        "matmul",
                        ps,
                        lhsT=vp[k][:, :, c * P : (c + 1) * P],
                        rhs=est[k],
                        start=(k == 0),
                        stop=(k == NJT // 2 - 1),
                        perf_mode=mybir.MatmulPerfMode.DoubleRow,
                    )
                if c == 0:
                    em("tensor", "matmul", den_ps, lhsT=ones_c_s, rhs=dacc,
                       start=True, stop=True)
                    with nc.allow_low_precision(reason="f32r rounding of 1/den"):
                        for q in range(4):
                            em("vector", "reciprocal",
                               out=r_s[:, q * P : (q + 1) * P],
                               in_=den_ps[:, q * P : (q + 1) * P])
                elif c == 1:
                    rb_chunk(0)
                    rb_chunk(1)
                elif c == 2:
                    rb_chunk(2)
                    rb_chunk(3)
            em("vector", "tensor_copy", out=rb_s, in_=rb_ps)
            for c in range(NCH):
                em("vector", "tensor_mul", out=h_s[c], in0=ps_c[c], in1=rb_s)
            # proj + bias' + residual (straight from the resident raw x)
            for o in range(NCH):
                ps = ps_main.tile([P, FD], f32, tag="ps", name="mps")
                for c in range(NCH):
                    em(
                        "tensor",
                        "matmul",
                        ps,
                        lhsT=wp_s[c][:, o * P : (o + 1) * P],
                        rhs=h_s[c],
                        start=(c == 0),
                        stop=(c == NCH - 1),
                    )
                o_s = ost.tile([P, FD], f32, tag="o_s", name="o_s")
                em(
                    "vector",
                    "scalar_tensor_tensor",
                    out=o_s,
                    in0=ps,
                    scalar=bp2_s[:, o : o + 1],
                    in1=xh[o][:, i0 : i0 + FD],
                    op0=OP.add,
                    op1=OP.add,
                )
                em(
                    "sync",
                    "dma_start",
                    out=out_d[o * P : (o + 1) * P, i0 : i0 + FD],
                    in_=o_s,
                )
        em.plant_tail()

    from concourse import mybir as _mybir

    deficit = redistribute_waits(nc, em, _mybir)
    return nc, em, deficit


_BUILT_MAP = {}


def get_built(has_bq=False):
    if has_bq not in _BUILT_MAP:
        needed = {}
        deficit = None
        for attempt in range(8):
            nc, em, deficit = _build_nc(dict(needed), has_bq=has_bq)
            if not deficit:
                break
            for key, n in deficit.items():
                needed[key] = max(needed.get(key, 0), n)
        else:
            raise RuntimeError(f"spare-wait fixpoint did not converge: {deficit}")
        from concourse import mybir

        bad = check_wait_budget(nc, em, mybir)
        if bad:
            raise RuntimeError(f"instructions over wait budget: {bad[:10]}")
        _BUILT_MAP[has_bq] = nc
    return _BUILT_MAP[has_bq]


def _host_prep(x, gamma, beta, wq, bq, wk, bk, wv, bv, wp, bp):
    import ml_dtypes

    f = np.float32
    bf = ml_dtypes.bfloat16

    def t128(v):  # [512] -> [128, 4] with element (p, t) = v[t*128 + p]
        return np.ascontiguousarray(v.reshape(NCH, P).T.astype(f))

    ind_g = np.zeros((P, GPC), f)
    ind_g[np.arange(P), np.arange(P) // CPG] = 1.0
    ind_b = np.ascontiguousarray(ind_g.T)
    bp2 = wp.astype(f) @ bv.astype(f) + bp.astype(f)
    # M = Wk.T @ Wq folds the q and k GEMMs into one (scores are bilinear in
    # hn); lhsT layout needs MT = M.T. bk cancels in the softmax; bq needs
    # the u-correction path (zero in this problem).
    mt = (wq.astype(np.float64).T @ wk.astype(np.float64)).astype(f)
    u = (wk.astype(np.float64).T @ bq.astype(np.float64)).astype(f)
    wvT = wv.T.astype(f)
    wpT = wp.T.astype(f)
    wall = np.concatenate([mt, wvT, wpT], axis=1).astype(bf)  # [C, 3C]
    consts = np.concatenate(
        [t128(gamma), t128(beta), t128(bp2), t128(u), ind_g], axis=1
    )
    shared = {
        "wall": np.ascontiguousarray(wall),
        "consts": np.ascontiguousarray(consts),
        "ind_b": ind_b,
        "ones_col": np.ones((P, 1), bf),
        "ones_row": np.ones((1, P), f),
    }
    return [
        {
            "x": np.ascontiguousarray(x[b].astype(f)),
            "xb": np.ascontiguousarray(x[b].astype(f).astype(bf)),
            **shared,
        }
        for b in range(B)
    ], bool(
        np.any(bq != 0)
    )


def run(inputs, trace=False, **kw):
    from concourse.bass_utils import run_bass_kernel_spmd

    in_maps, has_bq = _host_prep(**{k: np.asarray(v) for k, v in inputs.items()})
    nc = get_built(has_bq=has_bq)
    res = run_bass_kernel_spmd(nc, in_maps, list(range(B)), trace=trace, **kw)
    out = np.stack([res.results[b]["out"] for b in range(B)]).astype(np.float32)
    return out, res


def kernel(**inputs):
    out, _ = run(inputs, trace=False)
    return out
